# revision 14
# baseline (speedup 1.0000x reference)
"""Bass/Trainium2 kernel for nn_CSPAN_80582176408287 (pooling).

Data-parallel over batch: 8 batches -> 8 NeuronCores, weights replicated.

Key algebraic restructurings (exact, up to fp reassociation):
  - scores = (X @ fp_w + fp_b) @ Q^T / sqrt(MD).  The fp_b term is constant
    per motif row and cancels in the row softmax, so scores = X @ Ws with
    Ws = fp_w @ Q^T / sqrt(MD)  (pf never materialized: 17 GFLOP -> 0.13).
  - mf = attn @ (X @ vp_w + vp_b) = (attn @ X) @ vp_w + vp_b since softmax
    rows sum to 1 (vf never materialized).
  - wa2_b / ca2_b shift all logits of their softmax rows uniformly and
    cancel; they are dropped.
  - window scores: ws = tanh(X @ wa1 + b) @ wa2 computed once per position
    (windows overlap 2x; reference recomputes per window).
  - wf^T (windowed weighted sums) via banded matmuls: window n covers rows
    [32n, 32n+64); an s-chunk of 128 rows intersects windows 4t-1..4t+3,
    so rhs is a (128 x 5) band read from a DRAM-scattered waw buffer and
    outputs accumulate into overlapping PSUM free-dim ranges.
"""

import math
from contextlib import ExitStack

import numpy as np

import concourse.bass as bass
import concourse.tile as tile
from concourse import bacc, mybir
from concourse.masks import make_identity

F32 = mybir.dt.float32
AF = mybir.ActivationFunctionType
ALU = mybir.AluOpType
AX = mybir.AxisListType

B, S, D = 8, 4096, 512
M, MD = 32, 512
IMP_H = 256
WIN, STRIDE = 64, 32
NW = 128          # 127 strided windows + duplicated tail window
AGG_H = 256
EPS = 1e-5
NT = S // 128     # 32 s-chunks of 128
KD = D // 128     # 4 d-chunks of 128
NEG = 30.0        # masked logits get -30 (exp -> ~1e-13; ref uses -inf)

INPUT_SPECS = [
    ("x", [S, D]), ("maskc", [128, NT]),
    ("ws_w", [D, M]),
    ("wa1_w", [D, AGG_H]), ("wa2_w", [AGG_H]),
    ("wmask", [NW, WIN]), ("wmb", [NW, WIN]), ("bandmask", [128, 5]),
    ("vp_w", [D, MD]), ("vp_b", [MD]),
    ("ln_g", [MD]), ("ln_b", [MD]),
    ("imp1_w", [MD, IMP_H]), ("imp1_b", [IMP_H]),
    ("imp2_w", [IMP_H]), ("imp2_b", [1]),
    ("mout_w", [MD, D]), ("mout_b", [D]),
    ("ca1_w", [D, AGG_H]), ("ca1_b", [AGG_H]),
    ("ca2_w", [AGG_H]),
    ("hout_w", [D, D]), ("hout_b", [D]),
]
OUTPUT_SPECS = [("attn_o", [M, S]), ("motif_o", [D]), ("hier_o", [D])]


def _ap(base, off_elems, dims):
    """Raw AP into base's tensor at base.offset + off_elems with dims [[step,count],...]."""
    return bass.AP(tensor=base.tensor, offset=base.offset + off_elems, ap=dims)


def _bcast(dram_ap, parts, n):
    """Read a DRAM vector (n,) replicated across `parts` partitions."""
    return _ap(dram_ap, 0, [[0, parts], [1, n]])


def build_body(ctx, tc, io):
    nc = tc.nc
    singles = ctx.enter_context(tc.tile_pool(name="singles", bufs=1))
    dram = ctx.enter_context(tc.tile_pool(name="dram", bufs=1, space="DRAM"))

    # ---------------- persistent SBUF tiles ----------------
    xs = singles.tile([128, NT, D], F32)       # X, s-major: [s%128, s//128, d]
    attnT = singles.tile([128, NT, M], F32)    # masked+normalized attn^T chunks
    ident = singles.tile([128, 128], F32)
    make_identity(nc, ident)

    ws_sb = singles.tile([128, KD, M], F32)    # Ws (512,32) d-chunked
    wa1_sb = singles.tile([128, KD, AGG_H], F32)
    wa2_sb = singles.tile([128, 2, 1], F32)
    mask_sp = singles.tile([128, NT], F32)     # mask, s-partition chunks
    wmask_sb = singles.tile([NW, WIN], F32)
    wmb_sb = singles.tile([NW, WIN], F32)
    band_sb = singles.tile([128, 5], F32)
    vpw_sb = singles.tile([128, KD, MD], F32)
    moutw_sb = singles.tile([128, KD, D], F32)
    houtw_sb = singles.tile([128, KD, D], F32)
    imp1w_sb = singles.tile([128, KD, IMP_H], F32)
    ca1w_sb = singles.tile([128, KD, AGG_H], F32)
    ca2w_sb = singles.tile([128, 2, 1], F32)
    vpbB = singles.tile([M, MD], F32)
    lngB = singles.tile([M, MD], F32)
    lnbB = singles.tile([M, MD], F32)
    imp1bB = singles.tile([M, IMP_H], F32)
    imp2wB = singles.tile([M, IMP_H], F32)
    imp2bB = singles.tile([M, 1], F32)
    ca1bB = singles.tile([128, AGG_H], F32)
    moutb_sb = singles.tile([1, D], F32)
    houtb_sb = singles.tile([1, D], F32)
    eps_sb = singles.tile([M, 1], F32)
    ones_sb = singles.tile([128, 1], F32)
    arecB = singles.tile([128, M], F32)        # 1/rowsum broadcast over partitions

    wspos_dram = dram.tile([S + 64], F32)
    arec_dram = dram.tile([M], F32)
    wawbuf = dram.tile([32 + NW * WIN + 32], F32)  # scattered waw, front pad 32

    dma = nc.sync.dma_start

    # ---------------- weight / constant loads ----------------
    nc.vector.memset(eps_sb, EPS)
    nc.vector.memset(ones_sb, 1.0)
    for kc in range(KD):
        dma(out=ws_sb[:, kc, :], in_=io["ws_w"][kc * 128:(kc + 1) * 128, :])
        dma(out=wa1_sb[:, kc, :], in_=io["wa1_w"][kc * 128:(kc + 1) * 128, :])
        dma(out=vpw_sb[:, kc, :], in_=io["vp_w"][kc * 128:(kc + 1) * 128, :])
        dma(out=moutw_sb[:, kc, :], in_=io["mout_w"][kc * 128:(kc + 1) * 128, :])
        dma(out=houtw_sb[:, kc, :], in_=io["hout_w"][kc * 128:(kc + 1) * 128, :])
        dma(out=imp1w_sb[:, kc, :], in_=io["imp1_w"][kc * 128:(kc + 1) * 128, :])
        dma(out=ca1w_sb[:, kc, :], in_=io["ca1_w"][kc * 128:(kc + 1) * 128, :])
    for h in range(2):
        dma(out=wa2_sb[:, h, :], in_=_ap(io["wa2_w"][:], h * 128, [[1, 128], [0, 1]]))
        dma(out=ca2w_sb[:, h, :], in_=_ap(io["ca2_w"][:], h * 128, [[1, 128], [0, 1]]))
    dma(out=mask_sp, in_=io["maskc"][:])
    dma(out=wmask_sb, in_=io["wmask"][:])
    dma(out=wmb_sb, in_=io["wmb"][:])
    dma(out=band_sb, in_=io["bandmask"][:])
    dma(out=vpbB, in_=_bcast(io["vp_b"][:], M, MD))
    dma(out=lngB, in_=_bcast(io["ln_g"][:], M, MD))
    dma(out=lnbB, in_=_bcast(io["ln_b"][:], M, MD))
    dma(out=imp1bB, in_=_bcast(io["imp1_b"][:], M, IMP_H))
    dma(out=imp2wB, in_=_bcast(io["imp2_w"][:], M, IMP_H))
    dma(out=imp2bB, in_=_bcast(io["imp2_b"][:], M, 1))
    dma(out=ca1bB, in_=_bcast(io["ca1_b"][:], 128, AGG_H))
    dma(out=moutb_sb, in_=_ap(io["mout_b"][:], 0, [[0, 1], [1, D]]))
    dma(out=houtb_sb, in_=_ap(io["hout_b"][:], 0, [[0, 1], [1, D]]))

    # X load: (4096, 512) -> (128, 32, 512), 8 DMAs of 1MB
    x_r = io["x"][:].rearrange("(t p) d -> p t d", p=128)
    for g in range(8):
        dma(out=xs[:, 4 * g:4 * g + 4, :], in_=x_r[:, 4 * g:4 * g + 4, :])

    # ---- phase 1-3 fused per 512-wide s-block: transpose, scores+softmax,
    #      attn^T, a1/tanh/ws_pos ----
    with tc.tile_pool(name="xtb", bufs=2) as xtb_pool, \
         tc.tile_pool(name="stg", bufs=2) as stg_pool, \
         tc.tile_pool(name="a1s", bufs=2) as a1s_pool, \
         tc.tile_pool(name="tp_psum", bufs=2, space="PSUM") as tpp, \
         tc.tile_pool(name="sc_psum", bufs=2, space="PSUM") as scp, \
         tc.tile_pool(name="a1_psum", bufs=2, space="PSUM") as a1p, \
         tc.tile_pool(name="ws_psum", bufs=2, space="PSUM") as wsp:
        for n in range(8):
            # X^T block (128, KD, 512) via PE transposes
            xtb = xtb_pool.tile([128, KD, 512], F32, tag="xtb")
            for kc in range(KD):
                pt = tpp.tile([128, 512], F32, tag="tp")
                for i in range(4):
                    nc.tensor.transpose(
                        pt[:, i * 128:(i + 1) * 128],
                        xs[:, 4 * n + i, kc * 128:(kc + 1) * 128], ident)
                if kc % 2 == 0:
                    nc.scalar.copy(out=xtb[:, kc, :], in_=pt[:])
                else:
                    nc.vector.tensor_copy(out=xtb[:, kc, :], in_=pt[:])

            # scores^T (m-part) for this block, exp, transpose, mask
            sc = scp.tile([M, 512], F32, tag="sc")
            for kc in range(KD):
                nc.tensor.matmul(sc[:], ws_sb[:, kc, :], xtb[:, kc, :],
                                 start=(kc == 0), stop=(kc == KD - 1))
            stg = stg_pool.tile([M, 512], F32, tag="stg")
            nc.scalar.activation(out=stg[:], in_=sc[:], func=AF.Exp)
            pt2 = tpp.tile([128, 512], F32, tag="tp")
            for i in range(4):
                t = 4 * n + i
                nc.tensor.matmul(pt2[:, i * 32:i * 32 + M],
                                 stg[:, i * 128:(i + 1) * 128],
                                 ident[:M, :M], is_transpose=True)
                nc.vector.tensor_scalar_mul(out=attnT[:, t, :],
                                            in0=pt2[:, i * 32:i * 32 + M],
                                            scalar1=mask_sp[:, t:t + 1])

            # a1^T = wa1^T X^T per half, tanh, ws_pos chunk -> DRAM
            wps = wsp.tile([1, 512], F32, tag="wsp")
            for h in range(2):
                a1 = a1p.tile([128, 512], F32, tag="a1")
                for kc in range(KD):
                    nc.tensor.matmul(a1[:], wa1_sb[:, kc, h * 128:(h + 1) * 128],
                                     xtb[:, kc, :],
                                     start=(kc == 0), stop=(kc == KD - 1))
                a1s = a1s_pool.tile([128, 512], F32, tag="a1s")
                nc.scalar.activation(out=a1s[:], in_=a1[:], func=AF.Tanh)
                nc.tensor.matmul(wps[:], wa2_sb[:, h, :], a1s[:],
                                 start=(h == 0), stop=(h == 1))
            wss = stg_pool.tile([1, 512], F32, tag="wss")
            nc.scalar.copy(out=wss, in_=wps[:])
            dma(out=_ap(wspos_dram[:], n * 512, [[1, 512]]), in_=wss[:])

    # tail pad of position scores (window 127 reads past S; masked anyway)
    zt = singles.tile([1, 64], F32)
    nc.vector.memset(zt, 0.0)
    dma(out=_ap(wspos_dram[:], S, [[1, 64]]), in_=zt[:])

    # ---- softmax row sums over s (via ones matmul on attnT), normalize ----
    with tc.tile_pool(name="as_psum", bufs=1, space="PSUM") as asp:
        asum_p = asp.tile([1, M], F32)
        for t in range(NT):
            nc.tensor.matmul(asum_p[:], ones_sb[:], attnT[:, t, :],
                             start=(t == 0), stop=(t == NT - 1))
        arec = singles.tile([1, M], F32)
        nc.vector.reciprocal(arec, asum_p[:])
    dma(out=arec_dram[:], in_=arec[:])
    dma(out=arecB, in_=_bcast(arec_dram[:], 128, M))
    attnT_full = _ap(attnT, 0, [attnT.ap[0], [0, NT], [1, M]])
    nc.vector.tensor_mul(attnT, attnT, _ap(arecB, 0,
                                           [arecB.ap[0], [0, NT], [1, M]]))
    # attn output: attnT[s_local, t, m] -> attn_o[m, 128t + s_local]
    for t in range(NT):
        dma(out=_ap(io["attn_o"][:], 128 * t, [[1, 128], [S, M]]),
            in_=attnT[:, t, :])

    # ---------------- windowed softmax -> waw ----------------
    wsw = singles.tile([NW, WIN], F32)
    dma(out=wsw[:], in_=_ap(wspos_dram[:], 0, [[STRIDE, NW], [1, WIN]]))
    nc.vector.tensor_mul(wsw, wsw, wmask_sb)
    nc.vector.tensor_add(wsw, wsw, wmb_sb)      # masked -> -30
    waw = singles.tile([NW, WIN], F32)
    wsum = singles.tile([NW, 1], F32)
    nc.scalar.activation(out=waw, in_=wsw, func=AF.Exp, accum_out=wsum)
    wrec = singles.tile([NW, 1], F32)
    nc.vector.reciprocal(wrec, wsum)
    nc.vector.tensor_scalar_mul(waw, in0=waw, scalar1=wrec)

    # scatter waw[n,w] -> wawbuf[32 + 64n + w]; band-read Ball[s,t,j]
    dma(out=_ap(wawbuf[:], 32, [[WIN, NW], [1, WIN]]), in_=waw[:])
    ball = singles.tile([128, NT, 5], F32)
    for j in range(5):
        dma(out=ball[:, :, j], in_=_ap(wawbuf[:], 32 * j, [[1, 128], [256, NT]]))
    band_bc = bass.AP(tensor=band_sb.tensor, offset=band_sb.offset,
                      ap=[band_sb.ap[0], [0, NT], band_sb.ap[1]])
    nc.vector.tensor_mul(ball, ball, band_bc)

    # ---------------- amx = attn@X ; wf^T via banded matmuls ----------------
    # wf^T[d, n] accumulates in PSUM free dim (overlapping window ranges are
    # legal there; PE partition offsets are restricted to 0/32/64).
    wfT = singles.tile([128, KD, NW], F32)
    amx_s = singles.tile([M, MD], F32)
    with tc.tile_pool(name="amx_psum", bufs=1, space="PSUM") as amxp, \
         tc.tile_pool(name="wf_psum", bufs=1, space="PSUM") as wfp:
        amx = amxp.tile([M, MD], F32)
        for t in range(NT):
            nc.tensor.matmul(amx[:], attnT[:, t, :], xs[:, t, :],
                             start=(t == 0), stop=(t == NT - 1))
        nc.scalar.copy(out=amx_s[:], in_=amx[:])
        for kc in range(KD):
            wfp_t = wfp.tile([128, NW], F32, tag=f"wf{kc}")
            nc.vector.memset(wfp_t[:], 0.0)
            for t in range(NT):
                if t == 0:
                    nc.tensor.matmul(wfp_t[:, 0:4],
                                     xs[:, 0, kc * 128:(kc + 1) * 128],
                                     ball[:, 0, 1:5], start=False, stop=False,
                                     skip_group_check=True)
                else:
                    nc.tensor.matmul(wfp_t[:, 4 * t - 1:4 * t + 4],
                                     xs[:, t, kc * 128:(kc + 1) * 128],
                                     ball[:, t, :], start=False,
                                     stop=(t == NT - 1), skip_group_check=True)
            if kc % 2 == 0:
                nc.scalar.copy(out=wfT[:, kc, :], in_=wfp_t[:])
            else:
                nc.vector.tensor_copy(out=wfT[:, kc, :], in_=wfp_t[:])
    # duplicated tail window: wf[127] := wf[126] (free-dim column copy)
    nc.vector.tensor_copy(out=wfT[:, :, NW - 1:NW], in_=wfT[:, :, NW - 2:NW - 1])
    wf_s = singles.tile([NW, D], F32)

    # ---------------- motif + hier tails ----------------
    with tc.tile_pool(name="pt_psum", bufs=2, space="PSUM") as ptp, \
         tc.tile_pool(name="mid_psum", bufs=2, space="PSUM") as midp, \
         tc.tile_pool(name="vec_psum", bufs=2, space="PSUM") as vecp:

        def transpose_rows(src, ncols, dst):
            """src (r<=128, ncols*128) sbuf -> dst (128, ncols, r) via PE."""
            r = src.shape[0]
            for kc in range(ncols):
                pt = ptp.tile([128, 128], F32, tag="pt")
                nc.tensor.matmul(pt[:, :r], src[:, kc * 128:(kc + 1) * 128],
                                 ident[:r, :r], is_transpose=True)
                if kc % 2 == 0:
                    nc.scalar.copy(out=dst[:, kc, :], in_=pt[:, :r])
                else:
                    nc.vector.tensor_copy(out=dst[:, kc, :], in_=pt[:, :r])

        # mf = amx @ vp_w + vp_b
        amxT = singles.tile([128, KD, M], F32)
        transpose_rows(amx_s, KD, amxT)
        mf_p = midp.tile([M, MD], F32, tag="mid")
        for kc in range(KD):
            nc.tensor.matmul(mf_p[:], amxT[:, kc, :], vpw_sb[:, kc, :],
                             start=(kc == 0), stop=(kc == KD - 1))
        mf_s = singles.tile([M, MD], F32)
        nc.vector.tensor_add(mf_s, mf_p, vpbB)

        # LayerNorm over MD
        st6 = singles.tile([M, 6], F32)
        nc.vector.bn_stats(out=st6, in_=mf_s)
        mv = singles.tile([M, 2], F32)
        nc.vector.bn_aggr(out=mv, in_=st6)
        rstd = singles.tile([M, 1], F32)
        nc.scalar.activation(out=rstd, in_=mv[:, 1:2], func=AF.Sqrt, bias=eps_sb)
        nc.vector.reciprocal(rstd, rstd)
        mf_n = singles.tile([M, MD], F32)
        nc.vector.tensor_scalar(out=mf_n, in0=mf_s, scalar1=mv[:, 0:1],
                                scalar2=rstd, op0=ALU.subtract, op1=ALU.mult)
        nc.vector.tensor_mul(mf_n, mf_n, lngB)
        nc.vector.tensor_add(mf_n, mf_n, lnbB)

        # importance MLP -> iw = sigmoid(imp2(relu(imp1(mf_n))))
        mfnT = singles.tile([128, KD, M], F32)
        transpose_rows(mf_n, KD, mfnT)
        h1_p = midp.tile([M, IMP_H], F32, tag="mid")
        for kc in range(KD):
            nc.tensor.matmul(h1_p[:], mfnT[:, kc, :], imp1w_sb[:, kc, :],
                             start=(kc == 0), stop=(kc == KD - 1))
        h1 = singles.tile([M, IMP_H], F32)
        nc.vector.tensor_add(h1, h1_p, imp1bB)
        nc.scalar.activation(out=h1, in_=h1, func=AF.Relu)
        nc.vector.tensor_mul(h1, h1, imp2wB)
        iw = singles.tile([M, 1], F32)
        nc.vector.reduce_sum(out=iw, in_=h1, axis=AX.X)
        nc.vector.tensor_add(iw, iw, imp2bB)
        nc.scalar.activation(out=iw, in_=iw, func=AF.Sigmoid)

        # motif_out = (iw^T @ mf_n) @ mout_w + mout_b
        s1_p = vecp.tile([1, D], F32, tag="vec")
        nc.tensor.matmul(s1_p[:], iw[:], mf_n[:], start=True, stop=True)
        s1 = singles.tile([1, D], F32)
        nc.scalar.copy(out=s1, in_=s1_p[:])
        s1T = singles.tile([128, KD, 1], F32)
        transpose_rows(s1, KD, s1T)
        mo_p = vecp.tile([1, D], F32, tag="vec")
        for kc in range(KD):
            nc.tensor.matmul(mo_p[:], s1T[:, kc, :], moutw_sb[:, kc, :],
                             start=(kc == 0), stop=(kc == KD - 1))
        mo = singles.tile([1, D], F32)
        nc.vector.tensor_add(mo, mo_p, moutb_sb)
        dma(out=_ap(io["motif_o"][:], 0, [[0, 1], [1, D]]), in_=mo[:])

        # hier: h = relu(wf @ ca1 + b); cs = h @ ca2; cw softmax; out
        # wf_s (window-major) from native wfT via PE transposes
        for kc in range(KD):
            pt = ptp.tile([128, 128], F32, tag="pt")
            nc.tensor.matmul(pt[:], wfT[:, kc, :], ident[:], is_transpose=True)
            if kc % 2 == 0:
                nc.scalar.copy(out=wf_s[:, kc * 128:(kc + 1) * 128], in_=pt[:])
            else:
                nc.vector.tensor_copy(out=wf_s[:, kc * 128:(kc + 1) * 128],
                                      in_=pt[:])
        h_p = midp.tile([NW, AGG_H], F32, tag="midw")
        for kc in range(KD):
            nc.tensor.matmul(h_p[:], wfT[:, kc, :], ca1w_sb[:, kc, :],
                             start=(kc == 0), stop=(kc == KD - 1))
        hh = singles.tile([NW, AGG_H], F32)
        nc.vector.tensor_add(hh, h_p, ca1bB)
        nc.scalar.activation(out=hh, in_=hh, func=AF.Relu)
        hhT = singles.tile([128, 2, NW], F32)
        transpose_rows(hh, 2, hhT)
        cs_p = ptp.tile([128, 128], F32, tag="pt")
        for kc in range(2):
            nc.tensor.matmul(cs_p[:, 0:1], hhT[:, kc, :], ca2w_sb[:, kc, :],
                             start=(kc == 0), stop=(kc == 1))
        ecs = singles.tile([NW, 1], F32)
        nc.scalar.activation(out=ecs, in_=cs_p[:, 0:1], func=AF.Exp)
        se_p = vecp.tile([1, D], F32, tag="vec")
        nc.tensor.matmul(se_p[:, 0:1], ecs[:], ones_sb[:], start=True, stop=True)
        hg_p = vecp.tile([1, D], F32, tag="vec")
        nc.tensor.matmul(hg_p[:], ecs[:], wf_s[:], start=True, stop=True)
        srec = singles.tile([1, 1], F32)
        nc.vector.reciprocal(srec, se_p[:, 0:1])
        hg = singles.tile([1, D], F32)
        nc.vector.tensor_scalar_mul(hg, in0=hg_p[:], scalar1=srec)
        hgT = singles.tile([128, KD, 1], F32)
        transpose_rows(hg, KD, hgT)
        ho_p = vecp.tile([1, D], F32, tag="vec")
        for kc in range(KD):
            nc.tensor.matmul(ho_p[:], hgT[:, kc, :], houtw_sb[:, kc, :],
                             start=(kc == 0), stop=(kc == KD - 1))
        ho = singles.tile([1, D], F32)
        nc.vector.tensor_add(ho, ho_p, houtb_sb)
        dma(out=_ap(io["hier_o"][:], 0, [[0, 1], [1, D]]), in_=ho[:])


def build_nc():
    nc = bacc.Bacc()
    io = {}
    for name, shp in INPUT_SPECS:
        io[name] = nc.dram_tensor(name, shp, F32, kind="ExternalInput")
    for name, shp in OUTPUT_SPECS:
        io[name] = nc.dram_tensor(name, shp, F32, kind="ExternalOutput")
    with ExitStack() as ctx:
        tc = ctx.enter_context(tile.TileContext(nc))
        build_body(ctx, tc, io)
    nc.compile()
    return nc


def _host_prep(inputs):
    f = lambda k: np.asarray(inputs[k], dtype=np.float32)
    ws_w = (f("fp_w") @ f("motif_queries").T / math.sqrt(MD)).astype(np.float32)
    mask = np.asarray(inputs["mask"]).astype(np.float32)     # (B, S)
    starts = list(range(0, S - WIN + 1, STRIDE))
    if len(starts) == 0 or S - len(starts) * STRIDE > 0:
        starts.append(max(0, S - WIN))
    starts = np.asarray(starts)                               # (NW,) == 128
    idx = starts[:, None] + np.arange(WIN)[None, :]           # (NW, WIN)
    wmask = mask[:, idx]                                      # (B, NW, WIN)
    wmb = (wmask - 1.0) * NEG
    # bandmask[s_local, j'] = 1 iff 0 <= s_local - 32*(j'-1) < 64
    sl = np.arange(128)[:, None]
    jp = np.arange(5)[None, :]
    bandmask = ((sl - 32 * (jp - 1) >= 0) & (sl - 32 * (jp - 1) < WIN))
    bandmask = bandmask.astype(np.float32)

    shared = {
        "ws_w": ws_w,
        "wa1_w": f("wa1_w"), "wa2_w": f("wa2_w").reshape(-1),
        "bandmask": bandmask,
        "vp_w": f("vp_w"), "vp_b": f("vp_b"),
        "ln_g": f("ln_g"), "ln_b": f("ln_b"),
        "imp1_w": f("imp1_w"), "imp1_b": f("imp1_b"),
        "imp2_w": f("imp2_w").reshape(-1), "imp2_b": f("imp2_b"),
        "mout_w": f("mout_w"), "mout_b": f("mout_b"),
        "ca1_w": f("ca1_w"), "ca1_b": f("ca1_b"),
        "ca2_w": f("ca2_w").reshape(-1),
        "hout_w": f("hout_w"), "hout_b": f("hout_b"),
    }
    feats = np.asarray(inputs["features"], dtype=np.float32)
    in_maps = []
    for b in range(B):
        m = dict(shared)
        m["x"] = np.ascontiguousarray(feats[b])
        # mask chunks: maskc[s_local, t] = mask[128t + s_local]
        m["maskc"] = np.ascontiguousarray(mask[b].reshape(NT, 128).T)
        m["wmask"] = np.ascontiguousarray(wmask[b])
        m["wmb"] = np.ascontiguousarray(wmb[b]).astype(np.float32)
        in_maps.append(m)
    return in_maps


_CACHE = {}


def kernel(**inputs):
    from concourse.bass_utils import run_bass_kernel_spmd
    if "nc" not in _CACHE:
        _CACHE["nc"] = build_nc()
    nc = _CACHE["nc"]
    in_maps = _host_prep(inputs)
    res = run_bass_kernel_spmd(nc, in_maps, core_ids=list(range(B)))
    motif = np.stack([res.results[b]["motif_o"] for b in range(B)])
    attn = np.stack([res.results[b]["attn_o"] for b in range(B)])
    hier = np.stack([res.results[b]["hier_o"] for b in range(B)])
    return motif, attn, hier
